# revision 60
# baseline (speedup 1.0000x reference)
"""BitFeedForward (BitNet b1.58 MLP) Trainium2 kernel.

Full computation:
    h = gelu(bitlinear(x, w1, b1));  out = bitlinear(h, w2, b2)
    bitlinear(x,w,b) = actquant(rmsnorm(x)) @ ternary(w).T + b

Sharding: pure data-parallel over the 16384 tokens -> 2048 tokens/core on
8 NeuronCores.  No collectives.  Each core holds full (quantized) weights.
The host wrapper passes the weights PRE-TRANSPOSED ([K, out] layout; pure
data movement -- all arithmetic incl. quantization stays on device), so
weight prep needs no on-device transposes at all (the xbar dma transpose
was probed on HW to be an exact logical transpose, so the activation-side
transposes compose with untransposed-loaded weights).

Key numerics:
  - the rmsnorm factor r cancels inside activation_quant's round():
    round(xn * 127/max|xn|) == round(x * 127/max|x|), so the pre-matmul
    quantization needs only amax -- the rsqrt chain feeds only the
    post-matmul dequant scale gamma = amax*r*mean|w|/127
  - quantized activations are integers in [-127,127]  -> exact in bf16
  - quantized weights are ternary {-1,0,1}            -> exact in bf16
  - matmuls run bf16 x bf16 with f32 PSUM accumulation -> integer-exact
  - bias applied post-PSUM on DVE as psum*gamma + b_broadcast (one
    scalar_tensor_tensor), so no matmul depends on the scale chain
  - round() implemented as fl(c*x + 1.5*2^23) - 1.5*2^23  (RNE, matches
    jnp.round); no ACT table-set switches (Copy/Abs/Gelu only)
  - h is stored bf16 (adds ~5e-3 rel err via quant-boundary flips; the
    c2 quant-dequant pair is self-consistent so no scale error)

Scheduling (engines have a 4-deep OoO wait window, so any op waiting on a
far-future dep head-of-line blocks its whole queue; shallow staging rings
latency-chain pipelines):
  - w1 prep: all 32 loads issue up-front into a transient 128KB/partition
    SBUF staging pool, stats from SBUF, quant writes straight into the
    bf16 weight tiles (no DMA in the chain); pool released and its
    address space reused by the token-loop pools
  - w2 stats stream during the w1 phase on the idle GpSimd queue; the w2
    quant pass overlaps tiles 0-1 (loads+round GpSimd, subcast ACT, clip
    DVE in place)
  - mm2 runs with a 2-tile lag (k2T triple-buffered) so TensorE
    alternates mm1(t)/mm2(t-2) gaplessly
  - all dma transposes issue from SP only: concurrent transposes from
    two queues were observed to corrupt data on HW
"""

import sys

for _p in ("/opt/trn_rl_repo",):
    if _p not in sys.path:
        sys.path.insert(0, _p)

from contextlib import ExitStack

import numpy as np

import concourse.bass as bass
import concourse.mybir as mybir
import concourse.tile as tile
from concourse.bass import ts

F32 = mybir.dt.float32
BF16 = mybir.dt.bfloat16
AF = mybir.ActivationFunctionType
ALU = mybir.AluOpType
AX = mybir.AxisListType.X

P = 128
DIM = 1024
INNER = 4096
N_CORES = 8
TOKENS = 4 * 4096
TOK_PER_CORE = TOKENS // N_CORES  # 2048
TT = TOK_PER_CORE // P  # 16 token tiles per core
CC = 1024

MMAGIC = 12582912.0  # 1.5 * 2**23 : RNE rounding magic for |x| < 2^22
QB = 127.0
EPS = 1e-5

_DONE = object()


def _split_dma_waits(nc):
    """walrus codegen only supports ONE sync wait on DMA pseudo-instructions
    (PSEUDO_DMA_DIRECT2D etc).  Tile's sem-assignment can attach several.
    Move all but one wait onto standalone EventSemaphore (add 0) instructions
    on the issuing engine, inserted right before the DMA -- semantically
    identical (engine-order wait), codegen-legal."""
    idc = 0
    for f in nc.m.functions:
        for bb in f.blocks:
            changed = False
            new = []
            for inst in bb.instructions:
                tn = type(inst).__name__
                si = inst.sync_info
                if (
                    tn != "InstEventSemaphore"
                    and si is not None
                    and len(si.on_wait) > 1
                ):
                    waits = list(si.on_wait)
                    for w in waits[:-1]:
                        idc += 1
                        e = mybir.InstEventSemaphore(
                            name=f"WSPLIT-{idc}",
                            sync_type="semaphore",
                            id=w.id,
                            update_mode="sem-add-imm",
                            update_value=0,
                        )
                        e.engine = inst.engine
                        e.sync_info = mybir.SyncInfo(on_wait=[w], on_update=[])
                        nc.register_instruction(e, overwrite=True)
                        new.append(e)
                    inst.sync_info = mybir.SyncInfo(
                        on_wait=[waits[-1]], on_update=list(si.on_update)
                    )
                    changed = True
                new.append(inst)
            if changed:
                bb.instructions = new


def build(nc: bass.Bass, n_ttiles: int = TT, af_act=None):
    if af_act is None:
        af_act = AF.Gelu
    toks = n_ttiles * P
    x_d = nc.dram_tensor("x", [toks, DIM], F32, kind="ExternalInput")
    # weights arrive pre-transposed from the host: [K, out]
    w1_d = nc.dram_tensor("w1t", [DIM, INNER], F32, kind="ExternalInput")
    b1_d = nc.dram_tensor("b1", [INNER], F32, kind="ExternalInput")
    w2_d = nc.dram_tensor("w2t", [INNER, DIM], F32, kind="ExternalInput")
    b2_d = nc.dram_tensor("b2", [DIM], F32, kind="ExternalInput")
    out_d = nc.dram_tensor("out", [toks, DIM], F32, kind="ExternalOutput")

    with tile.TileContext(nc) as tc, ExitStack() as ctx:
        consts1 = ctx.enter_context(tc.tile_pool(name="consts1", bufs=1))
        dram = ctx.enter_context(tc.tile_pool(name="dram", bufs=1, space="DRAM"))
        psA = ctx.enter_context(tc.tile_pool(name="psA", bufs=5, space="PSUM"))
        psB = ctx.enter_context(tc.tile_pool(name="psB", bufs=2, space="PSUM"))
        psS = ctx.enter_context(tc.tile_pool(name="psS", bufs=1, space="PSUM"))
        st_p = ctx.enter_context(tc.tile_pool(name="st", bufs=3))

        ones = consts1.tile([P, 1], F32)
        nc.vector.memset(ones, 1.0)
        # w1T[:, j, m]: K-slice j (128 rows), out-feature m (4096)
        w1T = consts1.tile([P, 8, INNER], BF16)
        scal = consts1.tile([P, 4], F32)  # bcast scalars: ws1, mwd1, ws2, mwd2
        dsc = dram.tile([1, 4], F32)

        # ---------------- helpers ----------------
        def _bcast(dram_sc, sb_dst, src):
            nc.sync.dma_start(dram_sc, src)
            nc.sync.dma_start(sb_dst, dram_sc.to_broadcast(list(sb_dst.shape)))

        def _rsqrt_newton(v, seed, iters, tg):
            r = st_p.tile([P, 1], F32, tag=f"rs_r{tg}")
            nc.vector.memset(r, seed)
            for _ in range(iters):
                rr = st_p.tile([P, 1], F32, tag=f"rs_rr{tg}")
                nc.vector.tensor_mul(rr, r, r)
                t = st_p.tile([P, 1], F32, tag=f"rs_t{tg}")
                nc.vector.scalar_tensor_tensor(
                    out=t, in0=rr, scalar=-0.5, in1=v, op0=ALU.mult, op1=ALU.mult
                )
                r2 = st_p.tile([P, 1], F32, tag=f"rs_r2{tg}")
                nc.vector.scalar_tensor_tensor(
                    out=r2, in0=t, scalar=1.5, in1=r, op0=ALU.add, op1=ALU.mult
                )
                r = r2
            return r

        def _chain_c(amax, tg):
            am = st_p.tile([P, 1], F32, tag=f"c_am{tg}")
            nc.vector.tensor_scalar(
                out=am, in0=amax, scalar1=EPS, scalar2=None, op0=ALU.max
            )
            rec = st_p.tile([P, 1], F32, tag=f"c_rec{tg}")
            nc.vector.reciprocal(rec, am)
            c = st_p.tile([P, 1], F32, tag=f"c_c{tg}")
            nc.vector.tensor_scalar(
                out=c, in0=rec, scalar1=QB, scalar2=None, op0=ALU.mult
            )
            return c

        def _chain_gamma(mv, amax, seed, iters, mwd_col, tg):
            v = st_p.tile([P, 1], F32, tag=f"g_v{tg}")
            nc.vector.tensor_scalar(
                out=v, in0=mv[:, 0:1], scalar1=mv[:, 0:1], scalar2=None, op0=ALU.mult
            )
            nc.vector.tensor_scalar(
                out=v, in0=v, scalar1=mv[:, 1:2], scalar2=EPS, op0=ALU.add, op1=ALU.add
            )
            r = _rsqrt_newton(v, seed, iters, tg)
            g = st_p.tile([P, 1], F32, tag=f"g_g{tg}")
            nc.vector.scalar_tensor_tensor(
                out=g,
                in0=amax,
                scalar=scal[:, mwd_col : mwd_col + 1],
                in1=r,
                op0=ALU.mult,
                op1=ALU.mult,
            )
            return g

        _P1 = {}

        def wscale(tg, n_elems, ws_col, mwd_col, dsc_off):
            partials = _P1[tg]
            psum_v = st_p.tile([P, 1], F32, tag=f"psumv{tg}")
            nc.vector.tensor_reduce(out=psum_v, in_=partials, axis=AX, op=ALU.add)
            tot = psS.tile([1, 1], F32, tag="tot")
            nc.tensor.matmul(tot, psum_v, ones[:, 0:1], start=True, stop=True)
            mean = st_p.tile([1, 1], F32, tag=f"mean{tg}")
            nc.scalar.activation(mean, tot, AF.Copy, bias=0.0, scale=1.0 / n_elems)
            mw = st_p.tile([1, 1], F32, tag=f"mw{tg}")
            nc.vector.tensor_scalar(
                out=mw, in0=mean, scalar1=EPS, scalar2=None, op0=ALU.max
            )
            wsv = st_p.tile([1, 1], F32, tag=f"wsv{tg}")
            nc.vector.reciprocal(wsv, mw)
            mwd = st_p.tile([1, 1], F32, tag=f"mwd{tg}")
            nc.vector.tensor_scalar(
                out=mwd, in0=mw, scalar1=1.0 / QB, scalar2=None, op0=ALU.mult
            )
            _bcast(dsc[0:1, dsc_off : dsc_off + 1], scal[:, ws_col : ws_col + 1], wsv)
            _bcast(
                dsc[0:1, dsc_off + 1 : dsc_off + 2],
                scal[:, mwd_col : mwd_col + 1],
                mwd,
            )

        # ---------------- w1 phase (single pass, no transposes) ------------
        # w2's stats stream concurrently on GpSimd + DVE/ACT.
        with tc.tile_pool(name="w1s", bufs=1) as w1s_p, tc.tile_pool(
            name="w2s", bufs=2
        ) as w2s_p:
            w1s = w1s_p.tile([P, 32, CC], F32)
            part1 = st_p.tile([P, 32], F32, tag="partw1")
            _P1["w1"] = part1
            for i in range(32):
                j, c = i // 4, i % 4
                nc.gpsimd.dma_start(w1s[:, i, :], w1_d[ts(j, P), ts(c, CC)])
            def w2stats():
                part2 = st_p.tile([P, 32], F32, tag="partw2")
                _P1["w2"] = part2
                for r in range(32):
                    ws2t = w2s_p.tile([P, CC], F32, tag="w2s")
                    nc.gpsimd.dma_start(ws2t, w2_d[ts(r, P), :])
                    # in-place |w| on ACT (tile is dead after the accum);
                    # keeps DVE free for the concurrent w1 quant
                    nc.scalar.activation(
                        ws2t,
                        ws2t,
                        AF.Abs,
                        bias=0.0,
                        scale=1.0,
                        accum_out=part2[:, r : r + 1],
                    )
                    yield

            g_w2s = w2stats()
            w2s_alive = True
            # w1 stats on DVE only (they hide under the 47us load stream).
            # The w2 stats stream interleaves into the quant loop below, NOT
            # here: starting it early makes its DMA compete with the w1 load
            # stream (measured 14us slower), and interleaving all of it
            # head-of-line blocks the quant's ACT ops (measured much worse).
            for i in range(32):
                nc.vector.tensor_reduce(
                    out=part1[:, i : i + 1],
                    in_=w1s[:, i, :],
                    axis=AX,
                    op=ALU.add,
                    apply_absolute_value=True,
                )
            wscale("w1", 32 * P * 1024, 0, 1, 0)
            ws1_b = scal[:, 0:1]
            # quant straight into w1T: round (in place) -> subcast -> clip;
            # w2 stats stream in between
            for i in range(32):
                j, c = i // 4, i % 4
                src = w1s[:, i, :]
                dst = w1T[:, j, ts(c, CC)]
                if i % 2 == 0:
                    nc.scalar.activation(src, src, AF.Copy, bias=MMAGIC, scale=ws1_b)
                    nc.vector.tensor_scalar(
                        out=dst, in0=src, scalar1=MMAGIC, scalar2=None, op0=ALU.subtract
                    )
                else:
                    nc.vector.tensor_scalar(
                        out=src,
                        in0=src,
                        scalar1=ws1_b,
                        scalar2=MMAGIC,
                        op0=ALU.mult,
                        op1=ALU.add,
                    )
                    nc.scalar.activation(dst, src, AF.Copy, bias=-MMAGIC, scale=1.0)
                nc.vector.tensor_scalar(
                    out=dst, in0=dst, scalar1=-1.0, scalar2=1.0, op0=ALU.max, op1=ALU.min
                )
                if w2s_alive:
                    w2s_alive = next(g_w2s, _DONE) is not _DONE
            while w2s_alive:
                w2s_alive = next(g_w2s, _DONE) is not _DONE
        wscale("w2", 8 * P * 4096, 2, 3, 2)

        # ---------------- token-loop pools (reuse w1s address space) -------
        consts2 = ctx.enter_context(tc.tile_pool(name="consts2", bufs=1))
        # w2T[:, r, m]: K-slice r of 32 (128 rows of w2t), out m (1024)
        w2T = consts2.tile([P, 32, DIM], BF16)
        b1bc = consts2.tile([P, INNER], BF16)
        b2bc = consts2.tile([P, DIM], BF16)

        xin_p = ctx.enter_context(tc.tile_pool(name="xin", bufs=2))
        wf_p = ctx.enter_context(tc.tile_pool(name="wf", bufs=3))
        sc_p = ctx.enter_context(tc.tile_pool(name="sc512", bufs=1))
        qb_p = ctx.enter_context(tc.tile_pool(name="qb", bufs=1))
        k1T_p = ctx.enter_context(tc.tile_pool(name="k1T", bufs=2))
        k2T_p = ctx.enter_context(tc.tile_pool(name="k2T", bufs=3))
        h_p = ctx.enter_context(tc.tile_pool(name="h", bufs=4))
        out_p = ctx.enter_context(tc.tile_pool(name="out", bufs=1))

        def bias_prep():
            b1v = b1_d.rearrange("(a c) -> a c", a=1)
            for n in range(4):
                stg = wf_p.tile([P, CC], F32, tag="wf")
                nc.sync.dma_start(stg, b1v[0:1, ts(n, CC)].to_broadcast([P, CC]))
                nc.vector.tensor_copy(b1bc[:, ts(n, CC)], stg)
            b2v = b2_d.rearrange("(a c) -> a c", a=1)
            stg = wf_p.tile([P, CC], F32, tag="wf")
            nc.sync.dma_start(stg, b2v[0:1, :].to_broadcast([P, DIM]))
            nc.vector.tensor_copy(b2bc, stg)

        def w2quant():
            ws2_b = scal[:, 2:3]
            for r in range(32):
                wf = wf_p.tile([P, CC], F32, tag="wf")
                nc.gpsimd.dma_start(wf, w2_d[ts(r, P), :])
                nc.gpsimd.tensor_scalar(
                    out=wf,
                    in0=wf,
                    scalar1=ws2_b,
                    scalar2=MMAGIC,
                    op0=ALU.mult,
                    op1=ALU.add,
                )  # round on the idle Pool engine
                dst = w2T[:, r, :]
                nc.scalar.activation(dst, wf, AF.Copy, bias=-MMAGIC, scale=1.0)
                nc.vector.tensor_scalar(
                    out=dst, in0=dst, scalar1=-1.0, scalar2=1.0, op0=ALU.max, op1=ALU.min
                )
                yield

        # ---------------- token-loop stages ----------------
        def emit_A(t):
            xin = xin_p.tile([P, DIM], F32, tag="x")
            nc.sync.dma_start(xin, x_d[ts(t, P), :])
            amax = st_p.tile([P, 1], F32, tag="amax")
            nc.vector.tensor_reduce(
                out=amax, in_=xin, axis=AX, op=ALU.max, apply_absolute_value=True
            )
            stat6 = st_p.tile([P, 2, 6], F32, tag="st6")
            xv = xin.rearrange("p (a b) -> p a b", a=2)
            for a in range(2):
                nc.vector.bn_stats(out=stat6[:, a, :], in_=xv[:, a, :])
            mv = st_p.tile([P, 2], F32, tag="mv")
            nc.vector.bn_aggr(out=mv, in_=stat6)
            c1 = _chain_c(amax, "1")
            g1 = _chain_gamma(mv, amax, 1.0, 4, 1, "1")
            k1 = qb_p.tile([P, DIM], BF16, tag="qb")
            for q in range(2):
                k1m = sc_p.tile([P, 512], F32, tag="sc")
                nc.scalar.activation(
                    k1m, xin[:, ts(q, 512)], AF.Copy, bias=MMAGIC, scale=c1
                )
                nc.vector.tensor_scalar(
                    out=k1[:, ts(q, 512)],
                    in0=k1m,
                    scalar1=MMAGIC,
                    scalar2=None,
                    op0=ALU.subtract,
                )
            k1T = k1T_p.tile([P, 8, P], BF16, tag="k1T")
            nc.sync.dma_start_transpose(k1T, k1)
            return k1T, g1

        def emit_B(t, k1T, g1, interleave=None):
            hch = []
            for _hi in range(4):
                h = h_p.tile([P, CC], BF16, tag="h")
                hch.append(h)
            hst6 = st_p.tile([P, 8, 6], F32, tag="hst6")
            hmax8 = st_p.tile([P, 8], F32, tag="hmax8")
            for n in range(8):
                ps = psA.tile([P, 512], F32, tag="ps1")
                for j in range(8):
                    nc.tensor.matmul(
                        ps,
                        k1T[:, j, :],
                        w1T[:, j, ts(n, 512)],
                        start=(j == 0),
                        stop=(j == 7),
                    )
                nc.vector.scalar_tensor_tensor(
                    out=ps,
                    in0=ps,
                    scalar=g1,
                    in1=b1bc[:, ts(n, 512)],
                    op0=ALU.mult,
                    op1=ALU.add,
                )
                h = hch[n // 2][:, (n % 2) * 512 : (n % 2) * 512 + 512]
                nc.scalar.activation(h, ps, af_act, bias=0.0, scale=1.0)
                nc.vector.bn_stats(out=hst6[:, n, :], in_=h)
                nc.vector.tensor_reduce(
                    out=hmax8[:, n : n + 1],
                    in_=h,
                    axis=AX,
                    op=ALU.max,
                    apply_absolute_value=True,
                )
                if interleave is not None:
                    interleave()
            return hch, hst6, hmax8

        def emit_C(t, hch, hst6, hmax8):
            mvh = st_p.tile([P, 2], F32, tag="mvh")
            nc.vector.bn_aggr(out=mvh, in_=hst6)
            amaxh = st_p.tile([P, 1], F32, tag="amaxh")
            nc.vector.tensor_reduce(out=amaxh, in_=hmax8, axis=AX, op=ALU.max)
            c2 = _chain_c(amaxh, "2")
            g2 = _chain_gamma(mvh, amaxh, 1.75, 5, 3, "2")
            k2T = k2T_p.tile([P, 4, 8, P], BF16, tag="k2T")
            for cc in range(4):
                k2c = qb_p.tile([P, CC], BF16, tag="qb")
                for q in range(2):
                    k2m = sc_p.tile([P, 512], F32, tag="sc")
                    # round on DVE, subcast on ACT: keeps the ACT queue
                    # short between consecutive tiles' gelus (psA recycle)
                    nc.vector.tensor_scalar(
                        out=k2m,
                        in0=hch[cc][:, ts(q, 512)],
                        scalar1=c2,
                        scalar2=MMAGIC,
                        op0=ALU.mult,
                        op1=ALU.add,
                    )
                    nc.scalar.activation(
                        k2c[:, ts(q, 512)], k2m, AF.Copy, bias=-MMAGIC, scale=1.0
                    )
                nc.sync.dma_start_transpose(k2T[:, cc, :, :], k2c)
            return k2T, g2

        def emit_D(t, k2T, g2):
            ot = out_p.tile([P, DIM], F32, tag="ot")
            for n in range(2):
                ps2 = psB.tile([P, 512], F32, tag="ps2")
                first = True
                for cc in range(4):
                    for j in range(8):
                        nc.tensor.matmul(
                            ps2,
                            k2T[:, cc, j, :],
                            w2T[:, 8 * cc + j, ts(n, 512)],
                            start=first,
                            stop=(cc == 3 and j == 7),
                        )
                        first = False
                nc.vector.scalar_tensor_tensor(
                    out=ot[:, ts(n, 512)],
                    in0=ps2,
                    scalar=g2,
                    in1=b2bc[:, ts(n, 512)],
                    op0=ALU.mult,
                    op1=ALU.add,
                )
            nc.sync.dma_start(out_d[ts(t, P), :], ot)

        # ---------------- schedule ----------------
        def drive(gen, steps):
            for _ in range(steps):
                if next(gen, _DONE) is _DONE:
                    return False
            return True

        A = {}
        A[0] = emit_A(0)
        A[1] = emit_A(1)
        bias_prep()  # after A: b1bc is not needed until B(0)'s first stt

        gens = {"g": w2quant(), "alive": True}

        def il_step():
            # 1 step per B-block: spreads the w2 quant's ACT subcasts over
            # tiles 0-3 (2/block congested ACT between tiles 0-1's gelus;
            # w2T completion is Pool-serial-bound either way)
            if gens["alive"]:
                gens["alive"] = drive(gens["g"], 1)

        D = {}
        for t in range(n_ttiles):
            hch, hst6, hmax8 = emit_B(
                t, *A[t], interleave=il_step if gens["alive"] else None
            )
            # A(t+1) here (not at iteration start): its DVE chain must not
            # sit ahead of B(t)'s psum-dequant stts (psA recycle -> PE gap),
            # but it still needs a ~20us lead before B(t+1) uses k1T(t+1)
            if 2 <= t + 1 < n_ttiles:
                A[t + 1] = emit_A(t + 1)
            k2T, g2 = emit_C(t, hch, hst6, hmax8)
            D[t] = (k2T, g2)
            if t == 2:
                while gens["alive"]:
                    il_step()
            if t >= 2:
                emit_D(t - 2, *D[t - 2])
        while gens["alive"]:
            il_step()
        emit_D(n_ttiles - 2, *D[n_ttiles - 2])
        emit_D(n_ttiles - 1, *D[n_ttiles - 1])

    _split_dma_waits(nc)
    return nc, x_d, out_d


_CACHE = {}


def _get_compiled(n_ttiles=TT):
    if n_ttiles not in _CACHE:
        nc = bass.Bass()
        build(nc, n_ttiles)
        nc.finalize()
        _CACHE[n_ttiles] = nc
    return _CACHE[n_ttiles]


def kernel(x, w1, b1, w2, b2, _trace=False, _tmpdir=None):
    from concourse import bass_utils

    nc = _get_compiled(TT)
    xf = np.ascontiguousarray(x.reshape(TOKENS, DIM).astype(np.float32))
    w1t = np.ascontiguousarray(w1.astype(np.float32).T)  # [1024, 4096]
    b1 = np.ascontiguousarray(b1.astype(np.float32))
    w2t = np.ascontiguousarray(w2.astype(np.float32).T)  # [4096, 1024]
    b2 = np.ascontiguousarray(b2.astype(np.float32))
    in_maps = [
        {
            "x": xf[c * TOK_PER_CORE : (c + 1) * TOK_PER_CORE],
            "w1t": w1t,
            "b1": b1,
            "w2t": w2t,
            "b2": b2,
        }
        for c in range(N_CORES)
    ]
    res = bass_utils.run_bass_kernel_spmd(
        nc,
        in_maps,
        core_ids=list(range(N_CORES)),
        trace=_trace,
        tmpdir=_tmpdir,
    )
    outs = [res.results[c]["out"] for c in range(N_CORES)]
    full = np.concatenate(outs, axis=0).reshape(4, 4096, DIM).astype(np.float32)
    if _trace:
        return full, res
    return full


if __name__ == "__main__":
    nc = bass.Bass()
    build(nc, 4)
    nc.finalize()
    print("build+compile OK")
